# revision 24
# baseline (speedup 1.0000x reference)
"""LMU kernel for Trainium2, 8-core data-parallel.

Math (per batch b, with x[b] in [D, L] layout):
  u[b]    = relu(W_u @ x[b] + b_u)                              [1, L]
  m[b]    = H @ Toep(u[b])        (causal conv via Toeplitz)    [D, L]
  h[b]    = relu(W_h[:, :D] @ m[b] + W_h[:, D:] @ x[b] + b_h)   [D, L]
  y[b]    = BN(conv_w @ h[b] + conv_b)                          [D, L]

Device-side folds (host precomputes, O(params) only):
  F      = (W_h[:, :D] @ H).T, row-flipped  -> single K=128 contraction
           against the (flipped) Toeplitz of u
  C'     = (inv * conv_w).T, bias' = (conv_b - mean) * inv + beta   (BN fold)

v2 design notes (all matmul operands bf16, PSUM fp32):
  - x is cast to bf16 and packed on the HOST into [KC, 128, BPC, L]
    (partition-major), so every x-tile DMA is 128 descriptors of 1KB
    contiguous reads and there are NO on-device casts.  The baseline's
    DVE cast chain was the cause of mid-kernel PE stalls (u-matmuls
    waiting on casts).
  - The output is written bf16 in the same packed layout and upcast to
    f32 on the host (adds ~2e-4 rel err, 10x under the gate); halves
    the out DMA bytes.
  - Weights are host-packed so each staging transfer is one fully
    contiguous 128-descriptor DMA (>=1.5KB per partition run).
  - PE program order front-loads independent work: u(0), u(1),
    j0-x(block0), u(2), j1-x, j2-x, then the block-0 Toeplitz closes.
    All three early u->DRAM->Toeplitz round-trips overlap dense
    matmuls, so the PE never stalls and the HAM clock-gate ramps once.
  - Steady state: u(cb+2) issued at the top of block cb on the gpsimd
    SWDGE queue (latency-tolerant); out-DMAs ride the vector queue;
    the scalar queue keeps only ACTs + the latency-critical block-0
    u chain.
"""

import os
import numpy as np

import concourse.bass as bass
import concourse.mybir as mybir
from concourse import bacc
from concourse.tile import TileContext
from concourse.bass_utils import run_bass_kernel_spmd

B, D, L = 256, 768, 128
NCORES = 8
BPC = B // NCORES          # batches per core
NB = 4                     # batches per column block
NCB = BPC // NB            # column blocks per core
NCOL = NB * L              # 512 columns per block
KC = D // 128              # 6 chunks of 128 over the D dim
THETA = 128.0
BN_EPS = 1e-5

XRUN = BPC * L             # per-partition elements of packed x / out
WRUN = 3 * KC * 128        # per-partition elements of one whx half

TRACE = False
LAST_EXEC_NS = None

_H_CACHE = None
_NC_CACHE = None


def _impulse_response():
    """Replicates the reference's H = impulse response [D, L], on CPU."""
    global _H_CACHE
    if _H_CACHE is not None:
        return _H_CACHE
    import jax
    import jax.numpy as jnp
    from jax.scipy.linalg import expm

    cpu = jax.devices("cpu")[0]
    with jax.default_device(cpu):
        Q = np.arange(D, dtype=np.float32)
        R = ((2.0 * Q + 1.0) / THETA)[:, None]
        i, j = np.meshgrid(Q, Q, indexing="ij")
        A = (np.where(i < j, -1.0, (-1.0) ** (i - j + 1)).astype(np.float32)) * R
        Bm = (((-1.0) ** Q)[:, None]).astype(np.float32) * R
        Maug = np.zeros((D + 1, D + 1), dtype=np.float32)
        Maug[:D, :D] = A
        Maug[:D, D:] = Bm
        E = expm(jnp.asarray(Maug))
        Ad = E[:D, :D]
        Bd = E[:D, D:]

        def step(Apow, _):
            return Ad @ Apow, (Apow @ Bd)[:, 0]

        _, H = jax.lax.scan(step, jnp.eye(D, dtype=jnp.float32), None, length=L)
        _H_CACHE = np.asarray(H).T.astype(np.float32)  # [D, L]
    return _H_CACHE


def _build_nc():
    """Builds the (static) 8-core SPMD Bass program."""
    f32 = mybir.dt.float32
    bf16 = mybir.dt.bfloat16
    f8 = mybir.dt.float8e4
    DRow = mybir.MatmulPerfMode.DoubleRow
    nc = bacc.Bacc("TRN2", target_bir_lowering=False, debug=False, num_devices=NCORES)

    # x / out packed [KC, 128, BPC, L]: contiguous 1KB per partition per tile
    x_d = nc.dram_tensor("x", [KC, 128, BPC, L], bf16, kind="ExternalInput").ap()
    # fp8 copy of x (scaled by 32) for the u matmuls, DoubleRow-packed:
    # [ii, p, q, b, l] = x[b, (2*ii+q)*128+p, l] * 32
    x8_d = nc.dram_tensor("x8", [3, 128, 2, BPC, L], f8, kind="ExternalInput").ap()
    # u weights, DoubleRow stationary [p, q, ii, m]: col m=0 holds
    # W_u[(2ii+q)*128+p] * 4096, cols 1-15 zero (16-col width satisfies the
    # dual-fp8 LDWEIGHTS step%16 ISA restriction)
    wu8_d = nc.dram_tensor("wu8", [128, 2, 3, 16], f8, kind="ExternalInput").ap()
    out_d = nc.dram_tensor("out", [KC, 128, BPC, L], bf16, kind="ExternalOutput").ap()
    # whx halves: [2, 128(p), 3(j), KC(i), 128(c)] - each half one contiguous DMA
    whx_d = nc.dram_tensor("whx", [2, 128, 3, KC, 128], bf16, kind="ExternalInput").ap()
    # ct: [128(p), KC(j), KC(i), 128(c)] - one contiguous DMA
    ct_d = nc.dram_tensor("ct", [128, KC, KC, 128], bf16, kind="ExternalInput").ap()
    f_d = nc.dram_tensor("fmat", [L, D], bf16, kind="ExternalInput").ap()
    wu8_dram = None  # (bf16 wu no longer needed; u runs in fp8 DoubleRow)
    vecs_d = nc.dram_tensor("vecs", [128, KC * 3], f32, kind="ExternalInput").ap()
    upad_d = nc.dram_tensor("upad", [BPC * 2 * L], bf16).ap()  # internal scratch

    Relu = mybir.ActivationFunctionType.Relu

    with TileContext(nc) as tc:
        with (
            tc.tile_pool(name="const", bufs=1) as const,
            tc.tile_pool(name="xpool", bufs=6) as xpool,
            tc.tile_pool(name="x8pool", bufs=6) as x8pool,
            tc.tile_pool(name="hpool", bufs=14) as hpool,
            tc.tile_pool(name="tpool", bufs=4) as tpool,
            tc.tile_pool(name="opool", bufs=10) as opool,
            tc.tile_pool(name="upool", bufs=3) as upool,
            tc.tile_pool(name="pu", bufs=2, space="PSUM") as pu,
            tc.tile_pool(name="p3", bufs=3, space="PSUM") as p3,
            tc.tile_pool(name="p4", bufs=3, space="PSUM") as p4,
        ):
            # ---- constant tiles ----
            whx_r = const.tile([128, KC, KC, 128], bf16)  # [p | j | i | c]
            ct_r = const.tile([128, KC, KC, 128], bf16)   # [p | j | i | c]
            f_r = const.tile([128, D], bf16)              # [t' part | d]
            wu8_r = const.tile([128, 2, 3, 16], f8)
            vecs_sb = const.tile([128, KC, 3], f32)       # b_h, bias', b_u
            zt = const.tile([128, 2 * BPC], bf16)

            # wu + vecs on scalar (tiny, gate the first matmul / first ACT)
            nc.scalar.dma_start(out=wu8_r[:], in_=wu8_d)
            nc.scalar.dma_start(out=vecs_sb[:], in_=vecs_d)

            def load_x(cb, split=False):
                """Load block cb's x as ONE batched DMA (one SP config,
                768 descriptors of 1KB).  Block 0 is split: chunks 0-1 on
                sync and 2-5 on scalar, so the two queues' very first
                transfers both feed u(0) and the PE starts ~2.5us sooner."""
                b0 = cb * NB
                xt = xpool.tile([128, KC, NCOL], bf16, tag="xt")
                parts = (((0, 2), nc.sync), ((2, KC), nc.sync)) if split \
                    else (((0, KC), nc.sync),)
                for (i0, i1), eng in parts:
                    eng.dma_start(
                        out=xt[:, i0:i1, :],
                        in_=bass.AP(
                            tensor=x_d.tensor,
                            offset=i0 * 128 * XRUN + b0 * L,
                            ap=[[XRUN, 128], [128 * XRUN, i1 - i0], [1, NCOL]],
                        ),
                    )
                return [xt[:, i, :] for i in range(KC)]

            def load_x8(cb):
                """Load block cb's fp8 DoubleRow-packed x (for the u
                matmuls): one DMA, 768 descriptors of 512B."""
                b0 = cb * NB
                x8t = x8pool.tile([128, 2, 3, NCOL], f8, tag="x8t")
                nc.sync.dma_start(
                    out=x8t[:],
                    in_=bass.AP(
                        tensor=x8_d.tensor,
                        offset=b0 * L,
                        ap=[[2 * BPC * L, 128], [BPC * L, 2],
                            [128 * 2 * BPC * L, 3], [1, NCOL]],
                    ),
                )
                return x8t

            def stage_whx_half(h):
                nc.scalar.dma_start(
                    out=whx_r[:, h * 3:(h + 1) * 3, :, :],
                    in_=bass.AP(
                        tensor=whx_d.tensor,
                        offset=h * 128 * WRUN,
                        ap=[[WRUN, 128], [1, WRUN]],
                    ),
                )

            def compute_u(cb, x8t, chain_eng):
                """u = relu(W_u @ x + b_u) -> upad scratch -> Toeplitz tile.
                fp8 DoubleRow: 3 matmuls of 256-contraction at the full
                216ns rate; psum row 0 carries u * 2^17 (x*32, wu*4096),
                descaled in the activation."""
                psu = pu.tile([16, NCOL], f32, tag="pu")
                for ii in range(3):
                    nc.tensor.matmul(psu[:], wu8_r[:, :, ii, :],
                                     x8t[:, :, ii, :], start=(ii == 0),
                                     stop=(ii == 2), perf_mode=DRow)
                u_sb = upool.tile([1, NCOL], bf16, tag="u")
                nc.scalar.activation(u_sb[:], psu[0:1, :], Relu,
                                     bias=vecs_sb[0:1, 0, 2:3],
                                     scale=2.0 ** -17)
                t_r = tpool.tile([128, NCOL], bf16, tag="tr")
                chain_eng.dma_start(
                    out=bass.AP(tensor=upad_d.tensor,
                                offset=cb * NB * 2 * L + L,
                                ap=[[2 * L, NB], [1, L]]),
                    in_=u_sb[:],
                )
                chain_eng.dma_start(
                    out=t_r[:],
                    in_=bass.AP(tensor=upad_d.tensor,
                                offset=cb * NB * 2 * L + 1,
                                ap=[[1, 128], [2 * L, NB], [1, L]]),
                )
                return t_r

            def step3(cb, xr, t_r, js, hs):
                for j in js:
                    ps3 = p3.tile([128, NCOL], f32, tag="ps3")
                    for i in range(KC):
                        nc.tensor.matmul(ps3[:], whx_r[:, j, i, :],
                                         xr[i], start=(i == 0), stop=False)
                    nc.tensor.matmul(ps3[:], f_r[:, j * 128:(j + 1) * 128], t_r[:],
                                     start=False, stop=True)
                    hj = hpool.tile([128, NCOL], bf16, tag="h")
                    nc.scalar.activation(hj[:], ps3[:], Relu,
                                         bias=vecs_sb[:, j, 0:1])
                    hs.append(hj)

            def step4(cb, hs):
                b0 = cb * NB
                for j in range(KC):
                    ps4 = p4.tile([128, NCOL], f32, tag="ps4")
                    for i in range(KC):
                        nc.tensor.matmul(ps4[:], ct_r[:, j, i, :],
                                         hs[i][:], start=(i == 0), stop=(i == KC - 1))
                    oj = opool.tile([128, NCOL], bf16, tag="o")
                    nc.vector.tensor_scalar_add(oj[:], ps4[:], vecs_sb[:, j, 1:2])
                    # alternate out-DMAs across sync/scalar (sync is idle
                    # once x staging ends; keeping gpsimd light shrinks its
                    # expensive end-of-kernel SWDGE drain)
                    eng = nc.sync if j % 2 == 0 else nc.scalar
                    eng.dma_start(
                        out=bass.AP(
                            tensor=out_d.tensor,
                            offset=j * 128 * XRUN + b0 * L,
                            ap=[[XRUN, 128], [1, NCOL]],
                        ),
                        in_=oj[:],
                    )

            # ---- prologue staging.  sync: x0(c0-1), x1..x4 (+ even-j
            # outs later).  scalar HWDGE, in need-order: wu, vecs,
            # x0(c2-5), whx h0, whx h1, f, ct, then u0 chain + ACTs +
            # odd-j outs.  gpsimd SWDGE stays LIGHT (upad-zero + steady u
            # chains only): big staging through SWDGE floods the shared
            # DMA engines with tiny descriptors and starves the HWDGE
            # queues.  Both HWDGE queues config in parallel from ~6.8us
            # (fixed runtime startup); u(0) starts at ~10.9us.
            x8r = {0: load_x8(0)}
            xr = {0: load_x(0, split=True)}
            # zero the upad scratch (pad halves stay zero forever)
            nc.vector.memset(zt[:], 0.0)
            nc.gpsimd.dma_start(
                out=bass.AP(tensor=upad_d.tensor, offset=0,
                            ap=[[1, BPC * 2 * L]]),
                in_=zt[:],
            )
            x8r[1] = load_x8(1)
            xr[1] = load_x(1)
            stage_whx_half(0)
            stage_whx_half(1)
            nc.scalar.dma_start(out=f_r[:], in_=f_d)
            x8r[2] = load_x8(2)
            xr[2] = load_x(2)
            x8r[3] = load_x8(3)
            xr[3] = load_x(3)

            # ---- block 0: front-load independent PE work so all three
            # early u -> DRAM -> Toeplitz round-trips overlap dense matmuls.
            # u0's chain configs sit on the scalar ring BEFORE ct, so ct's
            # 3.3us transfer (needed only at ~24us) cannot crowd out the
            # early x/weight transfers on the shared DMA engines.
            t = {0: compute_u(0, x8r[0], nc.scalar)}
            nc.scalar.dma_start(
                out=ct_r[:],
                in_=bass.AP(tensor=ct_d.tensor, offset=0,
                            ap=[[KC * KC * 128, 128], [1, KC * KC * 128]]),
            )
            ps3s = [p3.tile([128, NCOL], f32, tag="ps3", name=f"ps3e{j}")
                    for j in range(3)]
            for i in range(KC):
                nc.tensor.matmul(ps3s[0][:], whx_r[:, 0, i, :],
                                 xr[0][i], start=(i == 0), stop=False)
            t[1] = compute_u(1, x8r[1], nc.gpsimd)
            t[2] = compute_u(2, x8r[2], nc.gpsimd)
            for j in range(1, 3):
                for i in range(KC):
                    nc.tensor.matmul(ps3s[j][:], whx_r[:, j, i, :],
                                     xr[0][i], start=(i == 0), stop=False)
            hs = []
            for j in range(3):
                nc.tensor.matmul(ps3s[j][:], f_r[:, j * 128:(j + 1) * 128],
                                 t[0][:], start=False, stop=True)
                hj = hpool.tile([128, NCOL], bf16, tag="h")
                nc.scalar.activation(hj[:], ps3s[j][:], Relu,
                                     bias=vecs_sb[:, j, 0:1])
                hs.append(hj)
            step3(0, xr[0], t[0], [3, 4, 5], hs)
            x8r[4] = load_x8(4)
            xr[4] = load_x(4)
            step4(0, hs)

            # ---- steady state: x stays 4 blocks ahead; the u -> Toeplitz
            # chain for block cb+2 is issued at the top of block cb.
            for cb in range(1, NCB):
                if cb + 2 < NCB:
                    t[cb + 2] = compute_u(cb + 2, x8r[cb + 2], nc.gpsimd)
                if cb + 4 < NCB:
                    x8r[cb + 4] = load_x8(cb + 4)
                    xr[cb + 4] = load_x(cb + 4)
                hs = []
                step3(cb, xr[cb], t[cb], [0, 1, 2], hs)
                step3(cb, xr[cb], t[cb], [3, 4, 5], hs)
                step4(cb, hs)

    if not nc.is_finalized():
        nc.finalize()
    return nc


def _get_nc():
    global _NC_CACHE
    if _NC_CACHE is None:
        _NC_CACHE = _build_nc()
    return _NC_CACHE


def _ensure_ntff_hook():
    """Register the NTFF profile hook if the deployment lacks antenv.axon_hooks."""
    import sys
    import types
    try:
        from antenv.axon_hooks import get_axon_ntff_profile_hook  # noqa: F401
        return
    except ImportError:
        pass
    try:
        from trn_agent_boot.trn_boot import _ntff_profile_via_ctypes
        hook = _ntff_profile_via_ctypes("/opt/axon/libaxon_pjrt.so")
        mod = types.ModuleType("antenv.axon_hooks")
        mod.get_axon_ntff_profile_hook = lambda: hook
        mod.set_axon_ntff_profile_hook = lambda h: None
        import antenv
        sys.modules["antenv.axon_hooks"] = mod
        antenv.axon_hooks = mod
    except Exception:
        pass


def kernel(x, W_u, b_u, W_h, b_h, conv_w, conv_b, bn_gamma, bn_beta, bn_mean,
           bn_var):
    global LAST_EXEC_NS
    x = np.ascontiguousarray(np.asarray(x, dtype=np.float32))
    W_u = np.asarray(W_u, dtype=np.float64)
    b_u = np.asarray(b_u, dtype=np.float64)
    W_h = np.asarray(W_h, dtype=np.float64)
    b_h = np.asarray(b_h, dtype=np.float64)
    conv_w = np.asarray(conv_w, dtype=np.float64)
    conv_b = np.asarray(conv_b, dtype=np.float64)
    bn_gamma = np.asarray(bn_gamma, dtype=np.float64)
    bn_beta = np.asarray(bn_beta, dtype=np.float64)
    bn_mean = np.asarray(bn_mean, dtype=np.float64)
    bn_var = np.asarray(bn_var, dtype=np.float64)
    assert x.shape == (B, D, L)

    H = _impulse_response().astype(np.float64)  # [D, L]

    # host folds (O(params) only)
    F = (W_h[:, :D] @ H).T[::-1, :]                      # [L, D], row-flipped
    inv = bn_gamma / np.sqrt(bn_var + BN_EPS)
    Whx = W_h[:, D:]                                     # [D(o), D(k)]
    Cp = conv_w[:, :, 0] * inv[:, None]                  # [D(o), D(k)]
    # lhsT layout [p(k-part), j(out chunk), i(k chunk), c(out col)]
    whx_pjic = Whx.reshape(KC, 128, KC, 128).transpose(3, 0, 2, 1)
    whx_pack = np.ascontiguousarray(
        whx_pjic.reshape(128, 2, 3, KC, 128).transpose(1, 0, 2, 3, 4))
    ct_pack = np.ascontiguousarray(
        Cp.reshape(KC, 128, KC, 128).transpose(3, 0, 2, 1))
    bias2 = (conv_b - bn_mean) * inv + bn_beta
    # fp8 DoubleRow u weights [p, q, ii, m] (m=0 real, 1-15 zero), *4096
    wu8 = np.zeros((128, 2, 3, 16), np.float64)
    wu8[:, :, :, 0] = (W_u[0] * 4096.0).reshape(3, 2, 128).transpose(2, 1, 0)
    vecs = np.stack([b_h, bias2, np.full(D, b_u[0])], axis=1)  # [D, 3]
    vecs_pack = np.ascontiguousarray(
        vecs.reshape(KC, 128, 3).transpose(1, 0, 2).reshape(128, KC * 3))

    nc = _get_nc()
    import ml_dtypes
    bf = ml_dtypes.bfloat16
    e4 = ml_dtypes.float8_e4m3
    shared = {
        "whx": whx_pack.astype(bf),
        "ct": ct_pack.astype(bf),
        "fmat": np.ascontiguousarray(F).astype(bf),
        "wu8": np.clip(wu8, -240, 240).astype(e4),
        "vecs": vecs_pack.astype(np.float32),
    }
    in_maps = []
    for c in range(NCORES):
        m = dict(shared)
        xs = x[c * BPC:(c + 1) * BPC]                      # [BPC, D, L]
        m["x"] = np.ascontiguousarray(
            xs.reshape(BPC, KC, 128, L).transpose(1, 2, 0, 3)).astype(bf)
        m["x8"] = np.clip(np.ascontiguousarray(
            xs.reshape(BPC, 3, 2, 128, L).transpose(1, 3, 2, 0, 4)) * 32.0,
            -240, 240).astype(e4)
        in_maps.append(m)

    if TRACE:
        _ensure_ntff_hook()
    res = run_bass_kernel_spmd(nc, in_maps, list(range(NCORES)), trace=TRACE)
    LAST_EXEC_NS = res.exec_time_ns
    outs = []
    for c in range(NCORES):
        packed = np.asarray(res.results[c]["out"])         # [KC, 128, BPC, L]
        outs.append(packed.transpose(2, 0, 1, 3).reshape(BPC, D, L))
    return np.concatenate(outs, axis=0).astype(np.float32)


# revision 27
# speedup vs baseline: 1.0027x; 1.0027x over previous
"""LMU kernel for Trainium2, 8-core data-parallel.

Math (per batch b, with x[b] in [D, L] layout):
  u[b]    = relu(W_u @ x[b] + b_u)                              [1, L]
  m[b]    = H @ Toep(u[b])        (causal conv via Toeplitz)    [D, L]
  h[b]    = relu(W_h[:, :D] @ m[b] + W_h[:, D:] @ x[b] + b_h)   [D, L]
  y[b]    = BN(conv_w @ h[b] + conv_b)                          [D, L]

Device-side folds (host precomputes, O(params) only):
  F      = (W_h[:, :D] @ H).T, row-flipped  -> single K=128 contraction
           against the (flipped) Toeplitz of u
  C'     = (inv * conv_w).T, bias' = (conv_b - mean) * inv + beta   (BN fold)

v2 design notes (all matmul operands bf16, PSUM fp32):
  - x is cast to bf16 and packed on the HOST into [KC, 128, BPC, L]
    (partition-major), so every x-tile DMA is 128 descriptors of 1KB
    contiguous reads and there are NO on-device casts.  The baseline's
    DVE cast chain was the cause of mid-kernel PE stalls (u-matmuls
    waiting on casts).
  - The output is written bf16 in the same packed layout and upcast to
    f32 on the host (adds ~2e-4 rel err, 10x under the gate); halves
    the out DMA bytes.
  - Weights are host-packed so each staging transfer is one fully
    contiguous 128-descriptor DMA (>=1.5KB per partition run).
  - PE program order front-loads independent work: u(0), u(1),
    j0-x(block0), u(2), j1-x, j2-x, then the block-0 Toeplitz closes.
    All three early u->DRAM->Toeplitz round-trips overlap dense
    matmuls, so the PE never stalls and the HAM clock-gate ramps once.
  - Steady state: u(cb+2) issued at the top of block cb on the gpsimd
    SWDGE queue (latency-tolerant); out-DMAs ride the vector queue;
    the scalar queue keeps only ACTs + the latency-critical block-0
    u chain.
"""

import os
import numpy as np

import concourse.bass as bass
import concourse.mybir as mybir
from concourse import bacc
from concourse.tile import TileContext
from concourse.bass_utils import run_bass_kernel_spmd

B, D, L = 256, 768, 128
NCORES = 8
BPC = B // NCORES          # batches per core
NB = 4                     # batches per column block
NCB = BPC // NB            # column blocks per core
NCOL = NB * L              # 512 columns per block
KC = D // 128              # 6 chunks of 128 over the D dim
THETA = 128.0
BN_EPS = 1e-5

XRUN = BPC * L             # per-partition elements of packed x / out
WRUN = 3 * KC * 128        # per-partition elements of one whx half

TRACE = False
LAST_EXEC_NS = None

_H_CACHE = None
_NC_CACHE = None


def _impulse_response():
    """Replicates the reference's H = impulse response [D, L], on CPU."""
    global _H_CACHE
    if _H_CACHE is not None:
        return _H_CACHE
    import jax
    import jax.numpy as jnp
    from jax.scipy.linalg import expm

    cpu = jax.devices("cpu")[0]
    with jax.default_device(cpu):
        Q = np.arange(D, dtype=np.float32)
        R = ((2.0 * Q + 1.0) / THETA)[:, None]
        i, j = np.meshgrid(Q, Q, indexing="ij")
        A = (np.where(i < j, -1.0, (-1.0) ** (i - j + 1)).astype(np.float32)) * R
        Bm = (((-1.0) ** Q)[:, None]).astype(np.float32) * R
        Maug = np.zeros((D + 1, D + 1), dtype=np.float32)
        Maug[:D, :D] = A
        Maug[:D, D:] = Bm
        E = expm(jnp.asarray(Maug))
        Ad = E[:D, :D]
        Bd = E[:D, D:]

        def step(Apow, _):
            return Ad @ Apow, (Apow @ Bd)[:, 0]

        _, H = jax.lax.scan(step, jnp.eye(D, dtype=jnp.float32), None, length=L)
        _H_CACHE = np.asarray(H).T.astype(np.float32)  # [D, L]
    return _H_CACHE


def _build_nc():
    """Builds the (static) 8-core SPMD Bass program."""
    f32 = mybir.dt.float32
    bf16 = mybir.dt.bfloat16
    f8 = mybir.dt.float8e4
    DRow = mybir.MatmulPerfMode.DoubleRow
    nc = bacc.Bacc("TRN2", target_bir_lowering=False, debug=False, num_devices=NCORES)

    # x / out packed [KC, 128, BPC, L]: contiguous 1KB per partition per tile
    x_d = nc.dram_tensor("x", [KC, 128, BPC, L], bf16, kind="ExternalInput").ap()
    # u weights, DoubleRow stationary [p, q, ii, m]: col m=0 holds
    # W_u[(2ii+q)*128+p] * 4096, cols 1-15 zero (16-col width satisfies the
    # dual-fp8 LDWEIGHTS step%16 ISA restriction)
    wu8_d = nc.dram_tensor("wu8", [128, 2, 3, 16], f8, kind="ExternalInput").ap()
    out_d = nc.dram_tensor("out", [KC, 128, BPC, L], bf16, kind="ExternalOutput").ap()
    # whx halves: [2, 128(p), 3(j), KC(i), 128(c)] - each half one contiguous DMA
    whx_d = nc.dram_tensor("whx", [2, 128, 3, KC, 128], bf16, kind="ExternalInput").ap()
    # ct: [128(p), KC(j), KC(i), 128(c)] - one contiguous DMA
    ct_d = nc.dram_tensor("ct", [128, KC, KC, 128], bf16, kind="ExternalInput").ap()
    f_d = nc.dram_tensor("fmat", [L, D], bf16, kind="ExternalInput").ap()
    wu8_dram = None  # (bf16 wu no longer needed; u runs in fp8 DoubleRow)
    vecs_d = nc.dram_tensor("vecs", [128, KC * 3], f32, kind="ExternalInput").ap()
    upad_d = nc.dram_tensor("upad", [BPC * 2 * L], bf16).ap()  # internal scratch

    Relu = mybir.ActivationFunctionType.Relu

    with TileContext(nc) as tc:
        with (
            tc.tile_pool(name="const", bufs=1) as const,
            tc.tile_pool(name="xpool", bufs=6) as xpool,
            tc.tile_pool(name="x8pool", bufs=6) as x8pool,
            tc.tile_pool(name="hpool", bufs=14) as hpool,
            tc.tile_pool(name="tpool", bufs=4) as tpool,
            tc.tile_pool(name="opool", bufs=10) as opool,
            tc.tile_pool(name="upool", bufs=3) as upool,
            tc.tile_pool(name="pu", bufs=2, space="PSUM") as pu,
            tc.tile_pool(name="p3", bufs=3, space="PSUM") as p3,
            tc.tile_pool(name="p4", bufs=3, space="PSUM") as p4,
        ):
            # ---- constant tiles ----
            whx_r = const.tile([128, KC, KC, 128], bf16)  # [p | j | i | c]
            ct_r = const.tile([128, KC, KC, 128], bf16)   # [p | j | i | c]
            f_r = const.tile([128, D], bf16)              # [t' part | d]
            wu8_r = const.tile([128, 2, 3, 16], f8)
            vecs_sb = const.tile([128, KC, 3], f32)       # b_h, bias', b_u
            zt = const.tile([128, 2 * BPC], bf16)

            # wu + vecs on scalar (tiny, gate the first matmul / first ACT)
            nc.scalar.dma_start(out=wu8_r[:], in_=wu8_d)
            nc.scalar.dma_start(out=vecs_sb[:], in_=vecs_d)

            def load_x(cb, split=False):
                """Load block cb's x as ONE batched DMA (one SP config,
                768 descriptors of 1KB).  Block 0 is split: chunks 0-1 on
                sync and 2-5 on scalar, so the two queues' very first
                transfers both feed u(0) and the PE starts ~2.5us sooner."""
                b0 = cb * NB
                xt = xpool.tile([128, KC, NCOL], bf16, tag="xt")
                parts = (((0, 2), nc.sync), ((2, KC), nc.sync)) if split \
                    else (((0, KC), nc.sync),)
                for (i0, i1), eng in parts:
                    eng.dma_start(
                        out=xt[:, i0:i1, :],
                        in_=bass.AP(
                            tensor=x_d.tensor,
                            offset=i0 * 128 * XRUN + b0 * L,
                            ap=[[XRUN, 128], [128 * XRUN, i1 - i0], [1, NCOL]],
                        ),
                    )
                return xt, [xt[:, i, :] for i in range(KC)]

            def cast_x8(xt, ranges=((0, KC),)):
                """DVE-cast a bf16 x tile into the fp8 u-matmul copy
                (chunk pair (2ii, 2ii+1) = DoubleRow planes for k-tile ii;
                unscaled: x in +-6 is exact-enough in e4m3, sim rel err
                6.7e-3).  `ranges` splits the cast so early chunks unblock
                u(0) before the whole tile has landed."""
                x8t = x8pool.tile([128, KC, NCOL], f8, tag="x8t")
                for lo, hi in ranges:
                    nc.vector.tensor_copy(x8t[:, lo:hi, :], xt[:, lo:hi, :])
                return x8t

            def stage_whx_half(h):
                nc.scalar.dma_start(
                    out=whx_r[:, h * 3:(h + 1) * 3, :, :],
                    in_=bass.AP(
                        tensor=whx_d.tensor,
                        offset=h * 128 * WRUN,
                        ap=[[WRUN, 128], [1, WRUN]],
                    ),
                )

            def compute_u(cb, x8t, chain_eng):
                """u = relu(W_u @ x + b_u) -> upad scratch -> Toeplitz tile.
                fp8 DoubleRow: 3 matmuls of 256-contraction at the full
                216ns rate; psum row 0 carries u * 2^17 (x*32, wu*4096),
                descaled in the activation."""
                psu = pu.tile([16, NCOL], f32, tag="pu")
                for ii in range(3):
                    nc.tensor.matmul(psu[:], wu8_r[:, :, ii, :],
                                     x8t[:, 2 * ii:2 * ii + 2, :],
                                     start=(ii == 0), stop=(ii == 2),
                                     perf_mode=DRow)
                u_sb = upool.tile([1, NCOL], bf16, tag="u")
                nc.scalar.activation(u_sb[:], psu[0:1, :], Relu,
                                     bias=vecs_sb[0:1, 0, 2:3],
                                     scale=2.0 ** -12)
                t_r = tpool.tile([128, NCOL], bf16, tag="tr")
                chain_eng.dma_start(
                    out=bass.AP(tensor=upad_d.tensor,
                                offset=cb * NB * 2 * L + L,
                                ap=[[2 * L, NB], [1, L]]),
                    in_=u_sb[:],
                )
                chain_eng.dma_start(
                    out=t_r[:],
                    in_=bass.AP(tensor=upad_d.tensor,
                                offset=cb * NB * 2 * L + 1,
                                ap=[[1, 128], [2 * L, NB], [1, L]]),
                )
                return t_r

            def step3(cb, xr, t_r, js, hs):
                for j in js:
                    ps3 = p3.tile([128, NCOL], f32, tag="ps3")
                    for i in range(KC):
                        nc.tensor.matmul(ps3[:], whx_r[:, j, i, :],
                                         xr[i], start=(i == 0), stop=False)
                    nc.tensor.matmul(ps3[:], f_r[:, j * 128:(j + 1) * 128], t_r[:],
                                     start=False, stop=True)
                    hj = hpool.tile([128, NCOL], bf16, tag="h")
                    nc.scalar.activation(hj[:], ps3[:], Relu,
                                         bias=vecs_sb[:, j, 0:1])
                    hs.append(hj)

            def step4(cb, hs):
                b0 = cb * NB
                for j in range(KC):
                    ps4 = p4.tile([128, NCOL], f32, tag="ps4")
                    for i in range(KC):
                        nc.tensor.matmul(ps4[:], ct_r[:, j, i, :],
                                         hs[i][:], start=(i == 0), stop=(i == KC - 1))
                    oj = opool.tile([128, NCOL], bf16, tag="o")
                    nc.vector.tensor_scalar_add(oj[:], ps4[:], vecs_sb[:, j, 1:2])
                    # alternate out-DMAs across sync/scalar (sync is idle
                    # once x staging ends; keeping gpsimd light shrinks its
                    # expensive end-of-kernel SWDGE drain).  The very last
                    # store is split across BOTH queues so the end-of-kernel
                    # drain is half a transfer.
                    if cb == NCB - 1 and j == KC - 1:
                        for (c0, c1), eng in (((0, NCOL // 2), nc.sync),
                                              ((NCOL // 2, NCOL), nc.scalar)):
                            eng.dma_start(
                                out=bass.AP(
                                    tensor=out_d.tensor,
                                    offset=j * 128 * XRUN + b0 * L + c0,
                                    ap=[[XRUN, 128], [1, c1 - c0]],
                                ),
                                in_=oj[:, c0:c1],
                            )
                    else:
                        eng = nc.sync if j % 2 == 0 else nc.scalar
                        eng.dma_start(
                            out=bass.AP(
                                tensor=out_d.tensor,
                                offset=j * 128 * XRUN + b0 * L,
                                ap=[[XRUN, 128], [1, NCOL]],
                            ),
                            in_=oj[:],
                        )

            # ---- prologue staging.  sync: x0(c0-1), x1..x4 (+ even-j
            # outs later).  scalar HWDGE, in need-order: wu, vecs,
            # x0(c2-5), whx h0, whx h1, f, ct, then u0 chain + ACTs +
            # odd-j outs.  gpsimd SWDGE stays LIGHT (upad-zero + steady u
            # chains only): big staging through SWDGE floods the shared
            # DMA engines with tiny descriptors and starves the HWDGE
            # queues.  Both HWDGE queues config in parallel from ~6.8us
            # (fixed runtime startup); u(0) starts at ~10.9us.
            xts = {}
            xr = {}
            xts[0], xr[0] = load_x(0, split=True)
            x8r = {0: cast_x8(xts[0], ranges=((0, 2), (2, KC)))}
            # zero the upad scratch (pad halves stay zero forever)
            nc.vector.memset(zt[:], 0.0)
            nc.gpsimd.dma_start(
                out=bass.AP(tensor=upad_d.tensor, offset=0,
                            ap=[[1, BPC * 2 * L]]),
                in_=zt[:],
            )
            # ---- PE warm-up: ~18 dummy matmuls on zeroed tiles while the
            # first x transfers stream in.  The HAM clock gate needs ~3us
            # of continuous PE activity to reach 2.4GHz; doing it on junk
            # during the DMA head means the REAL matmuls start warm
            # (saves the ~3us cold-ramp excess).
            wm = const.tile([128, NCOL], bf16)
            nc.vector.memset(wm[:], 0.0)
            for g in range(3):
                psw = pu.tile([16, NCOL], f32, tag="pu", name=f"warm{g}")
                for i in range(6):
                    nc.tensor.matmul(psw[:], zt[:, 0:16], wm[:],
                                     start=(i == 0), stop=(i == 5))
            xts[1], xr[1] = load_x(1)
            x8r[1] = cast_x8(xts[1])
            stage_whx_half(0)
            stage_whx_half(1)
            nc.scalar.dma_start(out=f_r[:], in_=f_d)
            xts[2], xr[2] = load_x(2)
            x8r[2] = cast_x8(xts[2])
            xts[3], xr[3] = load_x(3)
            x8r[3] = cast_x8(xts[3])

            # ---- block 0: front-load independent PE work so all three
            # early u -> DRAM -> Toeplitz round-trips overlap dense matmuls.
            # u0's chain configs sit on the scalar ring BEFORE ct, so ct's
            # 3.3us transfer (needed only at ~24us) cannot crowd out the
            # early x/weight transfers on the shared DMA engines.
            t = {0: compute_u(0, x8r[0], nc.scalar)}
            nc.scalar.dma_start(
                out=ct_r[:],
                in_=bass.AP(tensor=ct_d.tensor, offset=0,
                            ap=[[KC * KC * 128, 128], [1, KC * KC * 128]]),
            )
            ps3s = [p3.tile([128, NCOL], f32, tag="ps3", name=f"ps3e{j}")
                    for j in range(3)]
            for i in range(KC):
                nc.tensor.matmul(ps3s[0][:], whx_r[:, 0, i, :],
                                 xr[0][i], start=(i == 0), stop=False)
            t[1] = compute_u(1, x8r[1], nc.gpsimd)
            t[2] = compute_u(2, x8r[2], nc.gpsimd)
            for j in range(1, 3):
                for i in range(KC):
                    nc.tensor.matmul(ps3s[j][:], whx_r[:, j, i, :],
                                     xr[0][i], start=(i == 0), stop=False)
            hs = []
            for j in range(3):
                nc.tensor.matmul(ps3s[j][:], f_r[:, j * 128:(j + 1) * 128],
                                 t[0][:], start=False, stop=True)
                hj = hpool.tile([128, NCOL], bf16, tag="h")
                nc.scalar.activation(hj[:], ps3s[j][:], Relu,
                                     bias=vecs_sb[:, j, 0:1])
                hs.append(hj)
            step3(0, xr[0], t[0], [3, 4, 5], hs)
            xts[4], xr[4] = load_x(4)
            x8r[4] = cast_x8(xts[4])
            step4(0, hs)

            # ---- steady state: x stays 4 blocks ahead; the u -> Toeplitz
            # chain for block cb+2 is issued at the top of block cb.
            for cb in range(1, NCB):
                if cb + 2 < NCB:
                    t[cb + 2] = compute_u(cb + 2, x8r[cb + 2], nc.gpsimd)
                if cb + 4 < NCB:
                    xts[cb + 4], xr[cb + 4] = load_x(cb + 4)
                    x8r[cb + 4] = cast_x8(xts[cb + 4])
                hs = []
                step3(cb, xr[cb], t[cb], [0, 1, 2], hs)
                step3(cb, xr[cb], t[cb], [3, 4, 5], hs)
                step4(cb, hs)

    if not nc.is_finalized():
        nc.finalize()
    return nc


def _get_nc():
    global _NC_CACHE
    if _NC_CACHE is None:
        _NC_CACHE = _build_nc()
    return _NC_CACHE


def _ensure_ntff_hook():
    """Register the NTFF profile hook if the deployment lacks antenv.axon_hooks."""
    import sys
    import types
    try:
        from antenv.axon_hooks import get_axon_ntff_profile_hook  # noqa: F401
        return
    except ImportError:
        pass
    try:
        from trn_agent_boot.trn_boot import _ntff_profile_via_ctypes
        hook = _ntff_profile_via_ctypes("/opt/axon/libaxon_pjrt.so")
        mod = types.ModuleType("antenv.axon_hooks")
        mod.get_axon_ntff_profile_hook = lambda: hook
        mod.set_axon_ntff_profile_hook = lambda h: None
        import antenv
        sys.modules["antenv.axon_hooks"] = mod
        antenv.axon_hooks = mod
    except Exception:
        pass


def kernel(x, W_u, b_u, W_h, b_h, conv_w, conv_b, bn_gamma, bn_beta, bn_mean,
           bn_var):
    global LAST_EXEC_NS
    x = np.ascontiguousarray(np.asarray(x, dtype=np.float32))
    W_u = np.asarray(W_u, dtype=np.float64)
    b_u = np.asarray(b_u, dtype=np.float64)
    W_h = np.asarray(W_h, dtype=np.float64)
    b_h = np.asarray(b_h, dtype=np.float64)
    conv_w = np.asarray(conv_w, dtype=np.float64)
    conv_b = np.asarray(conv_b, dtype=np.float64)
    bn_gamma = np.asarray(bn_gamma, dtype=np.float64)
    bn_beta = np.asarray(bn_beta, dtype=np.float64)
    bn_mean = np.asarray(bn_mean, dtype=np.float64)
    bn_var = np.asarray(bn_var, dtype=np.float64)
    assert x.shape == (B, D, L)

    H = _impulse_response().astype(np.float64)  # [D, L]

    # host folds (O(params) only)
    F = (W_h[:, :D] @ H).T[::-1, :]                      # [L, D], row-flipped
    inv = bn_gamma / np.sqrt(bn_var + BN_EPS)
    Whx = W_h[:, D:]                                     # [D(o), D(k)]
    Cp = conv_w[:, :, 0] * inv[:, None]                  # [D(o), D(k)]
    # lhsT layout [p(k-part), j(out chunk), i(k chunk), c(out col)]
    whx_pjic = Whx.reshape(KC, 128, KC, 128).transpose(3, 0, 2, 1)
    whx_pack = np.ascontiguousarray(
        whx_pjic.reshape(128, 2, 3, KC, 128).transpose(1, 0, 2, 3, 4))
    ct_pack = np.ascontiguousarray(
        Cp.reshape(KC, 128, KC, 128).transpose(3, 0, 2, 1))
    bias2 = (conv_b - bn_mean) * inv + bn_beta
    # fp8 DoubleRow u weights [p, q, ii, m] (m=0 real, 1-15 zero), *4096
    wu8 = np.zeros((128, 2, 3, 16), np.float64)
    wu8[:, :, :, 0] = (W_u[0] * 4096.0).reshape(3, 2, 128).transpose(2, 1, 0)
    vecs = np.stack([b_h, bias2, np.full(D, b_u[0])], axis=1)  # [D, 3]
    vecs_pack = np.ascontiguousarray(
        vecs.reshape(KC, 128, 3).transpose(1, 0, 2).reshape(128, KC * 3))

    nc = _get_nc()
    import ml_dtypes
    bf = ml_dtypes.bfloat16
    e4 = ml_dtypes.float8_e4m3
    shared = {
        "whx": whx_pack.astype(bf),
        "ct": ct_pack.astype(bf),
        "fmat": np.ascontiguousarray(F).astype(bf),
        "wu8": np.clip(wu8, -240, 240).astype(e4),
        "vecs": vecs_pack.astype(np.float32),
    }
    in_maps = []
    for c in range(NCORES):
        m = dict(shared)
        xs = x[c * BPC:(c + 1) * BPC]                      # [BPC, D, L]
        m["x"] = np.ascontiguousarray(
            xs.reshape(BPC, KC, 128, L).transpose(1, 2, 0, 3)).astype(bf)
        in_maps.append(m)

    if TRACE:
        _ensure_ntff_hook()
    res = run_bass_kernel_spmd(nc, in_maps, list(range(NCORES)), trace=TRACE)
    LAST_EXEC_NS = res.exec_time_ns
    outs = []
    for c in range(NCORES):
        packed = np.asarray(res.results[c]["out"])         # [KC, 128, BPC, L]
        outs.append(packed.transpose(2, 0, 1, 3).reshape(BPC, D, L))
    return np.concatenate(outs, axis=0).astype(np.float32)


# revision 28
# speedup vs baseline: 1.0165x; 1.0137x over previous
"""LMU kernel for Trainium2, 8-core data-parallel.

Math (per batch b, with x[b] in [D, L] layout):
  u[b]    = relu(W_u @ x[b] + b_u)                              [1, L]
  m[b]    = H @ Toep(u[b])        (causal conv via Toeplitz)    [D, L]
  h[b]    = relu(W_h[:, :D] @ m[b] + W_h[:, D:] @ x[b] + b_h)   [D, L]
  y[b]    = BN(conv_w @ h[b] + conv_b)                          [D, L]

Device-side folds (host precomputes, O(params) only):
  F      = (W_h[:, :D] @ H).T, row-flipped  -> single K=128 contraction
           against the (flipped) Toeplitz of u
  C'     = (inv * conv_w).T, bias' = (conv_b - mean) * inv + beta   (BN fold)

v2 design notes (all matmul operands bf16, PSUM fp32):
  - x is cast to bf16 and packed on the HOST into [KC, 128, BPC, L]
    (partition-major), so every x-tile DMA is 128 descriptors of 1KB
    contiguous reads and there are NO on-device casts.  The baseline's
    DVE cast chain was the cause of mid-kernel PE stalls (u-matmuls
    waiting on casts).
  - The output is written bf16 in the same packed layout and upcast to
    f32 on the host (adds ~2e-4 rel err, 10x under the gate); halves
    the out DMA bytes.
  - Weights are host-packed so each staging transfer is one fully
    contiguous 128-descriptor DMA (>=1.5KB per partition run).
  - PE program order front-loads independent work: u(0), u(1),
    j0-x(block0), u(2), j1-x, j2-x, then the block-0 Toeplitz closes.
    All three early u->DRAM->Toeplitz round-trips overlap dense
    matmuls, so the PE never stalls and the HAM clock-gate ramps once.
  - Steady state: u(cb+2) issued at the top of block cb on the gpsimd
    SWDGE queue (latency-tolerant); out-DMAs ride the vector queue;
    the scalar queue keeps only ACTs + the latency-critical block-0
    u chain.
"""

import os
import numpy as np

import concourse.bass as bass
import concourse.mybir as mybir
from concourse import bacc
from concourse.tile import TileContext
from concourse.bass_utils import run_bass_kernel_spmd

B, D, L = 256, 768, 128
NCORES = 8
BPC = B // NCORES          # batches per core
NB = 4                     # batches per column block
NCB = BPC // NB            # column blocks per core
NCOL = NB * L              # 512 columns per block
KC = D // 128              # 6 chunks of 128 over the D dim
THETA = 128.0
BN_EPS = 1e-5

XRUN = BPC * L             # per-partition elements of packed x / out
WRUN = 3 * KC * 128        # per-partition elements of one whx half

TRACE = False
LAST_EXEC_NS = None

_H_CACHE = None
_NC_CACHE = None


def _impulse_response():
    """Replicates the reference's H = impulse response [D, L], on CPU."""
    global _H_CACHE
    if _H_CACHE is not None:
        return _H_CACHE
    import jax
    import jax.numpy as jnp
    from jax.scipy.linalg import expm

    cpu = jax.devices("cpu")[0]
    with jax.default_device(cpu):
        Q = np.arange(D, dtype=np.float32)
        R = ((2.0 * Q + 1.0) / THETA)[:, None]
        i, j = np.meshgrid(Q, Q, indexing="ij")
        A = (np.where(i < j, -1.0, (-1.0) ** (i - j + 1)).astype(np.float32)) * R
        Bm = (((-1.0) ** Q)[:, None]).astype(np.float32) * R
        Maug = np.zeros((D + 1, D + 1), dtype=np.float32)
        Maug[:D, :D] = A
        Maug[:D, D:] = Bm
        E = expm(jnp.asarray(Maug))
        Ad = E[:D, :D]
        Bd = E[:D, D:]

        def step(Apow, _):
            return Ad @ Apow, (Apow @ Bd)[:, 0]

        _, H = jax.lax.scan(step, jnp.eye(D, dtype=jnp.float32), None, length=L)
        _H_CACHE = np.asarray(H).T.astype(np.float32)  # [D, L]
    return _H_CACHE


def _build_nc():
    """Builds the (static) 8-core SPMD Bass program."""
    f32 = mybir.dt.float32
    bf16 = mybir.dt.bfloat16
    f8 = mybir.dt.float8e4
    DRow = mybir.MatmulPerfMode.DoubleRow
    nc = bacc.Bacc("TRN2", target_bir_lowering=False, debug=False, num_devices=NCORES)

    # x / out packed [KC, 128, BPC, L]: contiguous 1KB per partition per tile
    x_d = nc.dram_tensor("x", [KC, 128, BPC, L], bf16, kind="ExternalInput").ap()
    # u weights, DoubleRow stationary [p, q, ii, m]: col m=0 holds
    # W_u[(2ii+q)*128+p] * 4096, cols 1-15 zero (16-col width satisfies the
    # dual-fp8 LDWEIGHTS step%16 ISA restriction)
    wu8_d = nc.dram_tensor("wu8", [128, 2, 3, 16], f8, kind="ExternalInput").ap()
    out_d = nc.dram_tensor("out", [KC, 128, BPC, L], bf16, kind="ExternalOutput").ap()
    # whx halves: [2, 128(p), 3(j), KC(i), 128(c)] - each half one contiguous DMA
    whx_d = nc.dram_tensor("whx", [2, 128, 3, KC, 128], bf16, kind="ExternalInput").ap()
    # ct: [128(p), KC(j), KC(i), 128(c)] - one contiguous DMA
    ct_d = nc.dram_tensor("ct", [128, KC, KC, 128], bf16, kind="ExternalInput").ap()
    f_d = nc.dram_tensor("fmat", [L, D], bf16, kind="ExternalInput").ap()
    wu_d = nc.dram_tensor("wu", [128, KC], bf16, kind="ExternalInput").ap()
    vecs_d = nc.dram_tensor("vecs", [128, KC * 3], f32, kind="ExternalInput").ap()
    upad_d = nc.dram_tensor("upad", [BPC * 2 * L], bf16).ap()  # internal scratch

    Relu = mybir.ActivationFunctionType.Relu

    with TileContext(nc) as tc:
        with (
            tc.tile_pool(name="const", bufs=1) as const,
            tc.tile_pool(name="xpool", bufs=6) as xpool,
            tc.tile_pool(name="x8pool", bufs=6) as x8pool,
            tc.tile_pool(name="hpool", bufs=14) as hpool,
            tc.tile_pool(name="tpool", bufs=4) as tpool,
            tc.tile_pool(name="opool", bufs=10) as opool,
            tc.tile_pool(name="upool", bufs=3) as upool,
            tc.tile_pool(name="pu", bufs=2, space="PSUM") as pu,
            tc.tile_pool(name="p3", bufs=3, space="PSUM") as p3,
            tc.tile_pool(name="p4", bufs=3, space="PSUM") as p4,
        ):
            # ---- constant tiles ----
            whx_r = const.tile([128, KC, KC, 128], bf16)  # [p | j | i | c]
            ct_r = const.tile([128, KC, KC, 128], bf16)   # [p | j | i | c]
            f_r = const.tile([128, D], bf16)              # [t' part | d]
            wu8_r = const.tile([128, 2, 3, 16], f8)
            wu_r = const.tile([128, KC], bf16)
            vecs_sb = const.tile([128, KC, 3], f32)       # b_h, bias', b_u
            zt = const.tile([128, 2 * BPC], bf16)

            # wu + vecs on scalar (tiny, gate the first matmul / first ACT)
            nc.scalar.dma_start(out=wu8_r[:], in_=wu8_d)
            nc.scalar.dma_start(out=wu_r[:], in_=wu_d)
            nc.scalar.dma_start(out=vecs_sb[:], in_=vecs_d)

            def load_x(cb, split=False):
                """Load block cb's x as ONE batched DMA (one SP config,
                768 descriptors of 1KB).  Block 0 is split: chunks 0-1 on
                sync and 2-5 on scalar, so the two queues' very first
                transfers both feed u(0) and the PE starts ~2.5us sooner."""
                b0 = cb * NB
                xt = xpool.tile([128, KC, NCOL], bf16, tag="xt")
                parts = (((0, 2), nc.sync), ((2, KC), nc.sync)) if split \
                    else (((0, KC), nc.sync),)
                for (i0, i1), eng in parts:
                    eng.dma_start(
                        out=xt[:, i0:i1, :],
                        in_=bass.AP(
                            tensor=x_d.tensor,
                            offset=i0 * 128 * XRUN + b0 * L,
                            ap=[[XRUN, 128], [128 * XRUN, i1 - i0], [1, NCOL]],
                        ),
                    )
                return xt, [xt[:, i, :] for i in range(KC)]

            def cast_x8(xt, ranges=((0, KC),)):
                """DVE-cast a bf16 x tile into the fp8 u-matmul copy
                (chunk pair (2ii, 2ii+1) = DoubleRow planes for k-tile ii;
                unscaled: x in +-6 is exact-enough in e4m3, sim rel err
                6.7e-3).  `ranges` splits the cast so early chunks unblock
                u(0) before the whole tile has landed."""
                x8t = x8pool.tile([128, KC, NCOL], f8, tag="x8t")
                for lo, hi in ranges:
                    nc.vector.tensor_copy(x8t[:, lo:hi, :], xt[:, lo:hi, :])
                return x8t

            def stage_whx_half(h):
                nc.scalar.dma_start(
                    out=whx_r[:, h * 3:(h + 1) * 3, :, :],
                    in_=bass.AP(
                        tensor=whx_d.tensor,
                        offset=h * 128 * WRUN,
                        ap=[[WRUN, 128], [1, WRUN]],
                    ),
                )

            def compute_u(cb, xin, chain_eng, dr):
                """u = relu(W_u @ x + b_u) -> upad scratch -> Toeplitz tile.
                dr=True: fp8 DoubleRow, 3 matmuls of 256-contraction (xin
                is the fp8 cast tile; psum = 4096*u, descaled in the ACT).
                dr=False: 6 bf16 matmuls (xin is the bf16 slice list) --
                used for the early blocks whose u gates the pipeline, so
                they never wait on a DVE cast."""
                psu = pu.tile([16, NCOL], f32, tag="pu")
                if dr:
                    for ii in range(3):
                        nc.tensor.matmul(psu[:], wu8_r[:, :, ii, :],
                                         xin[:, 2 * ii:2 * ii + 2, :],
                                         start=(ii == 0), stop=(ii == 2),
                                         perf_mode=DRow)
                else:
                    for i in range(KC):
                        nc.tensor.matmul(psu[0:1, :], wu_r[:, i:i + 1],
                                         xin[i], start=(i == 0),
                                         stop=(i == KC - 1))
                u_sb = upool.tile([1, NCOL], bf16, tag="u")
                nc.scalar.activation(u_sb[:], psu[0:1, :], Relu,
                                     bias=vecs_sb[0:1, 0, 2:3],
                                     scale=(2.0 ** -12 if dr else 1.0))
                t_r = tpool.tile([128, NCOL], bf16, tag="tr")
                chain_eng.dma_start(
                    out=bass.AP(tensor=upad_d.tensor,
                                offset=cb * NB * 2 * L + L,
                                ap=[[2 * L, NB], [1, L]]),
                    in_=u_sb[:],
                )
                chain_eng.dma_start(
                    out=t_r[:],
                    in_=bass.AP(tensor=upad_d.tensor,
                                offset=cb * NB * 2 * L + 1,
                                ap=[[1, 128], [2 * L, NB], [1, L]]),
                )
                return t_r

            def step3(cb, xr, t_r, js, hs):
                for j in js:
                    ps3 = p3.tile([128, NCOL], f32, tag="ps3")
                    for i in range(KC):
                        nc.tensor.matmul(ps3[:], whx_r[:, j, i, :],
                                         xr[i], start=(i == 0), stop=False)
                    nc.tensor.matmul(ps3[:], f_r[:, j * 128:(j + 1) * 128], t_r[:],
                                     start=False, stop=True)
                    hj = hpool.tile([128, NCOL], bf16, tag="h")
                    nc.scalar.activation(hj[:], ps3[:], Relu,
                                         bias=vecs_sb[:, j, 0:1])
                    hs.append(hj)

            def step4(cb, hs):
                b0 = cb * NB
                for j in range(KC):
                    ps4 = p4.tile([128, NCOL], f32, tag="ps4")
                    for i in range(KC):
                        nc.tensor.matmul(ps4[:], ct_r[:, j, i, :],
                                         hs[i][:], start=(i == 0), stop=(i == KC - 1))
                    oj = opool.tile([128, NCOL], bf16, tag="o")
                    nc.vector.tensor_scalar_add(oj[:], ps4[:], vecs_sb[:, j, 1:2])
                    # alternate out-DMAs across sync/scalar (sync is idle
                    # once x staging ends; keeping gpsimd light shrinks its
                    # expensive end-of-kernel SWDGE drain).  The very last
                    # store is split across BOTH queues so the end-of-kernel
                    # drain is half a transfer.
                    if cb == NCB - 1 and j == KC - 1:
                        for (c0, c1), eng in (((0, NCOL // 2), nc.sync),
                                              ((NCOL // 2, NCOL), nc.scalar)):
                            eng.dma_start(
                                out=bass.AP(
                                    tensor=out_d.tensor,
                                    offset=j * 128 * XRUN + b0 * L + c0,
                                    ap=[[XRUN, 128], [1, c1 - c0]],
                                ),
                                in_=oj[:, c0:c1],
                            )
                    else:
                        eng = nc.sync if j % 2 == 0 else nc.scalar
                        eng.dma_start(
                            out=bass.AP(
                                tensor=out_d.tensor,
                                offset=j * 128 * XRUN + b0 * L,
                                ap=[[XRUN, 128], [1, NCOL]],
                            ),
                            in_=oj[:],
                        )

            # ---- prologue staging.  sync: x0(c0-1), x1..x4 (+ even-j
            # outs later).  scalar HWDGE, in need-order: wu, vecs,
            # x0(c2-5), whx h0, whx h1, f, ct, then u0 chain + ACTs +
            # odd-j outs.  gpsimd SWDGE stays LIGHT (upad-zero + steady u
            # chains only): big staging through SWDGE floods the shared
            # DMA engines with tiny descriptors and starves the HWDGE
            # queues.  Both HWDGE queues config in parallel from ~6.8us
            # (fixed runtime startup); u(0) starts at ~10.9us.
            xts = {}
            xr = {}
            x8r = {}
            xts[0], xr[0] = load_x(0, split=True)
            # zero the upad scratch (pad halves stay zero forever)
            nc.vector.memset(zt[:], 0.0)
            nc.gpsimd.dma_start(
                out=bass.AP(tensor=upad_d.tensor, offset=0,
                            ap=[[1, BPC * 2 * L]]),
                in_=zt[:],
            )
            # ---- PE warm-up: ~18 dummy matmuls on zeroed tiles while the
            # first x transfers stream in.  The HAM clock gate needs ~3us
            # of continuous PE activity to reach 2.4GHz; doing it on junk
            # during the DMA head means the REAL matmuls start warm
            # (saves the ~3us cold-ramp excess).
            wm = const.tile([128, NCOL], bf16)
            nc.vector.memset(wm[:], 0.0)
            for g in range(3):
                psw = pu.tile([16, NCOL], f32, tag="pu", name=f"warm{g}")
                for i in range(6):
                    nc.tensor.matmul(psw[:], zt[:, 0:16], wm[:],
                                     start=(i == 0), stop=(i == 5))
            xts[1], xr[1] = load_x(1)
            stage_whx_half(0)
            stage_whx_half(1)
            nc.scalar.dma_start(out=f_r[:], in_=f_d)
            xts[2], xr[2] = load_x(2)
            xts[3], xr[3] = load_x(3)
            x8r[3] = cast_x8(xts[3])

            # ---- block 0: front-load independent PE work so all three
            # early u -> DRAM -> Toeplitz round-trips overlap dense matmuls.
            # u0's chain configs sit on the scalar ring BEFORE ct, so ct's
            # 3.3us transfer (needed only at ~24us) cannot crowd out the
            # early x/weight transfers on the shared DMA engines.
            t = {0: compute_u(0, xr[0], nc.scalar, dr=False)}
            nc.scalar.dma_start(
                out=ct_r[:],
                in_=bass.AP(tensor=ct_d.tensor, offset=0,
                            ap=[[KC * KC * 128, 128], [1, KC * KC * 128]]),
            )
            ps3s = [p3.tile([128, NCOL], f32, tag="ps3", name=f"ps3e{j}")
                    for j in range(3)]
            for i in range(KC):
                nc.tensor.matmul(ps3s[0][:], whx_r[:, 0, i, :],
                                 xr[0][i], start=(i == 0), stop=False)
            t[1] = compute_u(1, xr[1], nc.gpsimd, dr=False)
            t[2] = compute_u(2, xr[2], nc.gpsimd, dr=False)
            for j in range(1, 3):
                for i in range(KC):
                    nc.tensor.matmul(ps3s[j][:], whx_r[:, j, i, :],
                                     xr[0][i], start=(i == 0), stop=False)
            hs = []
            for j in range(3):
                nc.tensor.matmul(ps3s[j][:], f_r[:, j * 128:(j + 1) * 128],
                                 t[0][:], start=False, stop=True)
                hj = hpool.tile([128, NCOL], bf16, tag="h")
                nc.scalar.activation(hj[:], ps3s[j][:], Relu,
                                     bias=vecs_sb[:, j, 0:1])
                hs.append(hj)
            step3(0, xr[0], t[0], [3, 4, 5], hs)
            xts[4], xr[4] = load_x(4)
            step4(0, hs)

            # ---- steady state: x stays 4 blocks ahead; the u -> Toeplitz
            # chain for block cb+2 is issued at the top of block cb.
            for cb in range(1, NCB):
                if cb + 4 < NCB:
                    xts[cb + 4], xr[cb + 4] = load_x(cb + 4)
                if cb + 3 < NCB and cb + 3 >= 4:
                    # cast for the DR-u two blocks ahead of its use;
                    # x(cb+3) landed during block cb-1, so the DVE never
                    # stalls on it
                    x8r[cb + 3] = cast_x8(xts[cb + 3])
                if cb + 2 < NCB:
                    t[cb + 2] = compute_u(cb + 2, x8r[cb + 2], nc.gpsimd,
                                          dr=True)
                hs = []
                step3(cb, xr[cb], t[cb], [0, 1, 2], hs)
                step3(cb, xr[cb], t[cb], [3, 4, 5], hs)
                step4(cb, hs)

    if not nc.is_finalized():
        nc.finalize()
    return nc


def _get_nc():
    global _NC_CACHE
    if _NC_CACHE is None:
        _NC_CACHE = _build_nc()
    return _NC_CACHE


def _ensure_ntff_hook():
    """Register the NTFF profile hook if the deployment lacks antenv.axon_hooks."""
    import sys
    import types
    try:
        from antenv.axon_hooks import get_axon_ntff_profile_hook  # noqa: F401
        return
    except ImportError:
        pass
    try:
        from trn_agent_boot.trn_boot import _ntff_profile_via_ctypes
        hook = _ntff_profile_via_ctypes("/opt/axon/libaxon_pjrt.so")
        mod = types.ModuleType("antenv.axon_hooks")
        mod.get_axon_ntff_profile_hook = lambda: hook
        mod.set_axon_ntff_profile_hook = lambda h: None
        import antenv
        sys.modules["antenv.axon_hooks"] = mod
        antenv.axon_hooks = mod
    except Exception:
        pass


def kernel(x, W_u, b_u, W_h, b_h, conv_w, conv_b, bn_gamma, bn_beta, bn_mean,
           bn_var):
    global LAST_EXEC_NS
    x = np.ascontiguousarray(np.asarray(x, dtype=np.float32))
    W_u = np.asarray(W_u, dtype=np.float64)
    b_u = np.asarray(b_u, dtype=np.float64)
    W_h = np.asarray(W_h, dtype=np.float64)
    b_h = np.asarray(b_h, dtype=np.float64)
    conv_w = np.asarray(conv_w, dtype=np.float64)
    conv_b = np.asarray(conv_b, dtype=np.float64)
    bn_gamma = np.asarray(bn_gamma, dtype=np.float64)
    bn_beta = np.asarray(bn_beta, dtype=np.float64)
    bn_mean = np.asarray(bn_mean, dtype=np.float64)
    bn_var = np.asarray(bn_var, dtype=np.float64)
    assert x.shape == (B, D, L)

    H = _impulse_response().astype(np.float64)  # [D, L]

    # host folds (O(params) only)
    F = (W_h[:, :D] @ H).T[::-1, :]                      # [L, D], row-flipped
    inv = bn_gamma / np.sqrt(bn_var + BN_EPS)
    Whx = W_h[:, D:]                                     # [D(o), D(k)]
    Cp = conv_w[:, :, 0] * inv[:, None]                  # [D(o), D(k)]
    # lhsT layout [p(k-part), j(out chunk), i(k chunk), c(out col)]
    whx_pjic = Whx.reshape(KC, 128, KC, 128).transpose(3, 0, 2, 1)
    whx_pack = np.ascontiguousarray(
        whx_pjic.reshape(128, 2, 3, KC, 128).transpose(1, 0, 2, 3, 4))
    ct_pack = np.ascontiguousarray(
        Cp.reshape(KC, 128, KC, 128).transpose(3, 0, 2, 1))
    bias2 = (conv_b - bn_mean) * inv + bn_beta
    # fp8 DoubleRow u weights [p, q, ii, m] (m=0 real, 1-15 zero), *4096
    wu8 = np.zeros((128, 2, 3, 16), np.float64)
    wu8[:, :, :, 0] = (W_u[0] * 4096.0).reshape(3, 2, 128).transpose(2, 1, 0)
    vecs = np.stack([b_h, bias2, np.full(D, b_u[0])], axis=1)  # [D, 3]
    vecs_pack = np.ascontiguousarray(
        vecs.reshape(KC, 128, 3).transpose(1, 0, 2).reshape(128, KC * 3))

    nc = _get_nc()
    import ml_dtypes
    bf = ml_dtypes.bfloat16
    e4 = ml_dtypes.float8_e4m3
    shared = {
        "whx": whx_pack.astype(bf),
        "ct": ct_pack.astype(bf),
        "fmat": np.ascontiguousarray(F).astype(bf),
        "wu8": np.clip(wu8, -240, 240).astype(e4),
        "wu": np.ascontiguousarray(W_u[0].reshape(KC, 128).T).astype(bf),
        "vecs": vecs_pack.astype(np.float32),
    }
    in_maps = []
    for c in range(NCORES):
        m = dict(shared)
        xs = x[c * BPC:(c + 1) * BPC]                      # [BPC, D, L]
        m["x"] = np.ascontiguousarray(
            xs.reshape(BPC, KC, 128, L).transpose(1, 2, 0, 3)).astype(bf)
        in_maps.append(m)

    if TRACE:
        _ensure_ntff_hook()
    res = run_bass_kernel_spmd(nc, in_maps, list(range(NCORES)), trace=TRACE)
    LAST_EXEC_NS = res.exec_time_ns
    outs = []
    for c in range(NCORES):
        packed = np.asarray(res.results[c]["out"])         # [KC, 128, BPC, L]
        outs.append(packed.transpose(2, 0, 1, 3).reshape(BPC, D, L))
    return np.concatenate(outs, axis=0).astype(np.float32)
